# revision 31
# baseline (speedup 1.0000x reference)
"""CRAFT OHEM loss on 8 trn2 NeuronCores — data-parallel over batch.

Math: with uniform-random inputs, n_neg_total (≈0.25·N) is always far below
NEG_RATIO·n_pos (≈2.25·N), so the reference's OHEM top-k selects *all*
negatives and every branch of the loss reduces to masked global sums:

    pos  = (region_target > .5) | (affinity_target > .5)   [= max(rt,at) > .5]
    S_pos_r = Σ pos·(rp-rt)²    S_tot_r = Σ (rp-rt)²       (same for affinity)
    n_pos   = Σ pos             n_neg_tot = N - n_pos

    region_loss   = S_pos_r/n_pos + (S_tot_r - S_pos_r)/n_neg
    affinity_loss = S_pos_a/n_pos + (S_tot_a - S_pos_a)/n_neg

The kernel streams fp16 inputs (host-side dtype cast; quantization error on
the final losses is ~1e-4, far inside the 2e-2 gate) — halving the HBM
traffic that bounds this kernel.  Preds and targets are host-interleaved as
[P, 2, F] pairs so one DMA / one DVE sub covers both maps.  Per chunk the
engines split the work so every engine hides under the DMA stream
(DMA ~100%, ACT ~70%, DVE ~62%, Pool ~45%, PE ~38%):

    DVE : d = pa − ta  [2x mode], mx = max(ta_r, ta_a) [2x],
          m = (mx > 0.5)  [tensor_scalar, 4x mode]
    ACT : sq = d²  [one instr; accum → per-chunk C = Σ(sq_r+sq_a)]
    Pool: stot[cols] = Σ_p sq_r  [GPSIMD cross-partition reduce]
    PE  : G += mᵀ · [m | sq_r | sq_a]  (PSUM-accumulated Gram; its three
          128×128 diagonal blocks are n_pos / S_pos_r / S_pos_a)

Host: S_tot_r = Σstot, S_tot_a = ΣC − S_tot_r, diagonals from G, all in
float64; falls back to an exact numpy OHEM on the original fp32 inputs in
the (unreachable for this input distribution) case
n_neg_tot > NEG_RATIO·n_pos.
"""

import numpy as np

import concourse.bass as bass
import concourse.bacc as bacc
import concourse.mybir as mybir
from concourse.tile import TileContext
from concourse.bass_utils import run_bass_kernel_spmd

N_CORES = 8
B, H, W = 32, 640, 640
N_TOTAL = B * H * W                  # 13_107_200
PER_CORE = N_TOTAL // N_CORES        # 1_638_400
P = 128
F_TOT = PER_CORE // P                # 12_800
NEG_RATIO = 3.0
MM = 128                             # matmul stationary width

_F32 = mybir.dt.float32
_F16 = mybir.dt.float16


_CHUNKS = [512] * 24 + [384, 128]
_BUFS = (6, 4)


def _chunk_list(f_tot: int = F_TOT) -> list:
    """Chunks must be multiples of MM=128 (matmul subchunking).  The taper at
    the end shortens the serial drain chain (DVE→ACT→PE) after the last
    input DMA lands."""
    return list(_CHUNKS)


def build_nc() -> bass.Bass:
    chunks = _chunk_list()
    nchunk = len(chunks)
    assert sum(chunks) == F_TOT

    nc = bacc.Bacc(None)
    # preds / targets host-interleaved as [P, 2, F_TOT] (plane 0 = region,
    # plane 1 = affinity): one DMA + one DVE sub covers both maps.
    pa = nc.dram_tensor("preds", [P, 2, F_TOT], _F16, kind="ExternalInput")
    ta = nc.dram_tensor("targs", [P, 2, F_TOT], _F16, kind="ExternalInput")
    # gram: [p, h, j] = Σ_k m[k,p]·[m|sq_r|sq_a][k,h,j] over 128-col
    #       subchunks; the three 128×128 diagonals are n_pos / S_pos_r /
    #       S_pos_a.
    # gram out:
    #   [:, 0:384]          = G (masked Gram, diagonals = n_pos/S_pos_r/a)
    #   [:, 384:384+nchunk] = per-chunk ACT accum C_i = Σ(sq_r+sq_a)
    # stot out: per-column Σ_p sq_r (GPSIMD reduce) — total is S_tot_r;
    #   S_tot_a = ΣC − S_tot_r on the host.
    W_OUT = 3 * MM + nchunk
    g_out = nc.dram_tensor("gram", [P, W_OUT], _F32, kind="ExternalOutput")
    st_out = nc.dram_tensor("stot", [1, F_TOT], _F32, kind="ExternalOutput")

    SQ = mybir.ActivationFunctionType.Square
    IS_GT = mybir.AluOpType.is_gt
    MULT = mybir.AluOpType.mult

    n_mm = F_TOT // MM  # total matmul count (per map pair)

    with TileContext(nc) as tc:
        with tc.tile_pool(name="io", bufs=_BUFS[0]) as io, \
             tc.tile_pool(name="mid", bufs=_BUFS[1]) as mid, \
             tc.tile_pool(name="fix", bufs=1) as fix, \
             tc.tile_pool(name="ps", bufs=1, space="PSUM") as ps:
            g_sb = fix.tile([P, W_OUT], _F32)
            st = fix.tile([1, F_TOT], _F32)
            G = ps.tile([P, 3, MM], _F32)

            mm_idx = 0
            off = 0
            for i, f in enumerate(chunks):
                sl = bass.ds(off, f)
                off += f
                # targets first: the mask path (mx) only needs them
                ta_t = io.tile([P, 2, f], _F16, tag="ta")
                nc.sync.dma_start(out=ta_t[:], in_=ta[:, :, sl])
                pa_t = io.tile([P, 2, f], _F16, tag="pa")
                nc.sync.dma_start(out=pa_t[:], in_=pa[:, :, sl])

                # sub first: it unblocks the ACT square, the longest
                # downstream chain
                d = mid.tile([P, 2, f], _F16, tag="d")
                nc.vector.tensor_sub(d[:], pa_t[:], ta_t[:])

                mx = mid.tile([P, f], _F16, tag="mx")
                nc.vector.tensor_max(mx[:], ta_t[:, 0, :], ta_t[:, 1, :])

                msq = mid.tile([P, 3, f], _F16, tag="msq")
                nc.vector.tensor_scalar(msq[:, 0, :], mx[:], 0.5, None, IS_GT)

                nc.scalar.activation(
                    msq[:, 1:3, :], d[:, :, :], SQ,
                    accum_out=g_sb[:, 3 * MM + i : 3 * MM + i + 1],
                )

                nc.gpsimd.reduce_sum(
                    st[:, sl], msq[:, 1, :], axis=mybir.AxisListType.C
                )
                for s in range(0, f, MM):
                    nc.tensor.matmul(
                        G[:, :, :],
                        msq[:, 0, s : s + MM],
                        msq[:, :, s : s + MM],
                        start=(mm_idx == 0),
                        stop=(mm_idx == n_mm - 1),
                    )
                    mm_idx += 1

            nc.sync.dma_start(out=st_out[:], in_=st[:])
            nc.vector.tensor_copy(g_sb[:, : 3 * MM], G[:, :, :])
            nc.scalar.dma_start(out=g_out[:], in_=g_sb[:])
    nc.compile()
    return nc


_NC_CACHE: dict = {}


def _get_nc() -> bass.Bass:
    if "nc" not in _NC_CACHE:
        _NC_CACHE["nc"] = build_nc()
    return _NC_CACHE["nc"]


def _shard2(r16: np.ndarray, a16: np.ndarray, c: int) -> np.ndarray:
    """Interleave the region/affinity pair for core c as [P, 2, F_TOT]."""
    per_b = B // N_CORES
    r = r16.reshape(B, H * W)[c * per_b : (c + 1) * per_b].reshape(P, F_TOT)
    a = a16.reshape(B, H * W)[c * per_b : (c + 1) * per_b].reshape(P, F_TOT)
    return np.ascontiguousarray(np.stack([r, a], axis=1))


def _core_inputs(np_inputs: dict, c: int) -> dict:
    """Device input map for core c from full-size fp32 inputs (test harness)."""
    return {
        "preds": _shard2(np_inputs["region_pred"].astype(np.float16),
                         np_inputs["affinity_pred"].astype(np.float16), c),
        "targs": _shard2(np_inputs["region_target"].astype(np.float16),
                         np_inputs["affinity_target"].astype(np.float16), c),
    }


def _host_fallback_topk(region_pred, affinity_pred, region_target, affinity_target,
                        n_pos, n_neg):
    """Exact OHEM (reference semantics) on host — unreachable for uniform data."""
    rlm = (region_pred.astype(np.float64) - region_target.astype(np.float64)) ** 2
    alm = (affinity_pred.astype(np.float64) - affinity_target.astype(np.float64)) ** 2
    pos = (region_target > 0.5) | (affinity_target > 0.5)
    neg = ~pos
    comb = ((rlm + alm) * neg).reshape(-1)
    idx = np.argsort(-comb, kind="stable")[:n_neg]
    neg_r = rlm.reshape(-1)[idx].mean()
    neg_a = alm.reshape(-1)[idx].mean()
    pos_r = (rlm * pos).sum() / n_pos
    pos_a = (alm * pos).sum() / n_pos
    return pos_r + neg_r, pos_a + neg_a


def kernel(region_pred, affinity_pred, region_target, affinity_target):
    region_pred = np.asarray(region_pred, dtype=np.float32)
    affinity_pred = np.asarray(affinity_pred, dtype=np.float32)
    region_target = np.asarray(region_target, dtype=np.float32)
    affinity_target = np.asarray(affinity_target, dtype=np.float32)

    rp16 = region_pred.astype(np.float16)
    ap16 = affinity_pred.astype(np.float16)
    rt16 = region_target.astype(np.float16)
    at16 = affinity_target.astype(np.float16)

    nc = _get_nc()
    in_maps = [
        {
            "preds": _shard2(rp16, ap16, c),
            "targs": _shard2(rt16, at16, c),
        }
        for c in range(N_CORES)
    ]
    res = run_bass_kernel_spmd(nc, in_maps, list(range(N_CORES))).results

    nchunk = len(_chunk_list())
    S_pos_r = S_pos_a = S_tot_r = S_tot_a = n_pos_f = 0.0
    for c in range(N_CORES):
        out = res[c]["gram"].astype(np.float64)
        g = out[:, : 3 * MM].reshape(P, 3, MM)
        comb = out[:, 3 * MM : 3 * MM + nchunk].sum()
        tot_r = res[c]["stot"].astype(np.float64).sum()
        S_tot_r += tot_r
        S_tot_a += comb - tot_r
        n_pos_f += np.trace(g[:, 0, :])
        S_pos_r += np.trace(g[:, 1, :])
        S_pos_a += np.trace(g[:, 2, :])

    n_pos = int(round(n_pos_f))
    n_neg_tot = N_TOTAL - n_pos

    if n_pos == 0:
        region_loss = S_tot_r / N_TOTAL
        affinity_loss = S_tot_a / N_TOTAL
    else:
        pos_r = S_pos_r / n_pos
        pos_a = S_pos_a / n_pos
        n_neg = min(n_neg_tot, int(n_pos * NEG_RATIO))
        if n_neg == 0:
            region_loss, affinity_loss = pos_r, pos_a
        elif n_neg == n_neg_tot:
            region_loss = pos_r + (S_tot_r - S_pos_r) / n_neg
            affinity_loss = pos_a + (S_tot_a - S_pos_a) / n_neg
        else:
            region_loss, affinity_loss = _host_fallback_topk(
                region_pred, affinity_pred, region_target, affinity_target,
                n_pos, n_neg,
            )

    total = np.float32(region_loss + affinity_loss)
    return (total, np.float32(region_loss), np.float32(affinity_loss))


# revision 46
# speedup vs baseline: 1.0123x; 1.0123x over previous
"""CRAFT OHEM loss on 8 trn2 NeuronCores — data-parallel over batch.

Math: with uniform-random inputs, n_neg_total (≈0.25·N) is always far below
NEG_RATIO·n_pos (≈2.25·N), so the reference's OHEM top-k selects *all*
negatives and every branch of the loss reduces to masked global sums:

    pos  = (region_target > .5) | (affinity_target > .5)   [= max(rt,at) > .5]
    S_pos_r = Σ pos·(rp-rt)²    S_tot_r = Σ (rp-rt)²       (same for affinity)
    n_pos   = Σ pos             n_neg_tot = N - n_pos

    region_loss   = S_pos_r/n_pos + (S_tot_r - S_pos_r)/n_neg
    affinity_loss = S_pos_a/n_pos + (S_tot_a - S_pos_a)/n_neg

The kernel streams fp16 inputs (host-side dtype cast; quantization error on
the final losses is ~1e-4, far inside the 2e-2 gate) — halving the HBM
traffic that bounds this kernel.  Preds and targets are host-interleaved as
[P, 2, F] pairs so one DMA / one DVE sub covers both maps.  Per chunk the
engines split the work so every engine hides under the DMA stream
(DMA ~100%, ACT ~70%, DVE ~62%, Pool ~45%, PE ~38%):

    DVE : d = pa − ta  [2x mode], mx = max(ta_r, ta_a) [2x],
          m = (mx > 0.5)  [tensor_scalar, 4x mode]
    ACT : sq = d²  [one instr; accum → per-chunk C = Σ(sq_r+sq_a)]
    Pool: stot[cols] = Σ_p sq_r  [GPSIMD cross-partition reduce]
    PE  : G += mᵀ · [m | sq_r | sq_a]  (PSUM-accumulated Gram; its three
          128×128 diagonal blocks are n_pos / S_pos_r / S_pos_a)

The final (tiny) chunk's squares run on DVE instead of ACT, with extra Pool
reduces covering its sums, so ACT drops out of the end-of-stream drain
chain one chunk early.

Host: S_tot_r = Σstot, S_tot_a = ΣC − S_tot_r, diagonals from G, all in
float64; falls back to an exact numpy OHEM on the original fp32 inputs in
the (unreachable for this input distribution) case
n_neg_tot > NEG_RATIO·n_pos.
"""

import numpy as np

import concourse.bass as bass
import concourse.bacc as bacc
import concourse.mybir as mybir
from concourse.tile import TileContext
from concourse.bass_utils import run_bass_kernel_spmd

N_CORES = 8
B, H, W = 32, 640, 640
N_TOTAL = B * H * W                  # 13_107_200
PER_CORE = N_TOTAL // N_CORES        # 1_638_400
P = 128
F_TOT = PER_CORE // P                # 12_800
NEG_RATIO = 3.0
MM = 128                             # matmul stationary width

_F32 = mybir.dt.float32
_F16 = mybir.dt.float16


_CHUNKS = [512] * 24 + [384, 128]
_BUFS = (6, 4)
_DVE_TAIL = 1          # how many final chunks bypass ACT (DVE squares them)


def _chunk_list(f_tot: int = F_TOT) -> list:
    """Chunks must be multiples of MM=128 (matmul subchunking).  The taper at
    the end shortens the serial drain chain (DVE→ACT→PE) after the last
    input DMA lands."""
    return list(_CHUNKS)


def build_nc() -> bass.Bass:
    chunks = _chunk_list()
    nchunk = len(chunks)
    assert sum(chunks) == F_TOT

    nc = bacc.Bacc(None)
    # preds / targets host-interleaved as [P, 2, F_TOT] (plane 0 = region,
    # plane 1 = affinity): one DMA + one DVE sub covers both maps.
    pa = nc.dram_tensor("preds", [P, 2, F_TOT], _F16, kind="ExternalInput")
    ta = nc.dram_tensor("targs", [P, 2, F_TOT], _F16, kind="ExternalInput")
    # gram: [p, h, j] = Σ_k m[k,p]·[m|sq_r|sq_a][k,h,j] over 128-col
    #       subchunks; the three 128×128 diagonals are n_pos / S_pos_r /
    #       S_pos_a.
    # gram out:
    #   [:, 0:384]          = G (masked Gram, diagonals = n_pos/S_pos_r/a)
    #   [:, 384:384+nchunk] = per-chunk ACT accum C_i = Σ(sq_r+sq_a)
    # stot out: per-column Σ_p sq_r (GPSIMD reduce) — total is S_tot_r;
    #   S_tot_a = ΣC − S_tot_r on the host.
    W_OUT = 3 * MM + nchunk
    g_out = nc.dram_tensor("gram", [P, W_OUT], _F16, kind="ExternalOutput")
    st_out = nc.dram_tensor("stot", [1, F_TOT + sum(chunks[-_DVE_TAIL:])], _F32, kind="ExternalOutput")
    # final _DVE_TAIL chunks skip ACT (DVE squares them); their Σsq_a comes
    # from extra Pool reduces so ACT exits the drain chain early
    tail_f = sum(chunks[-_DVE_TAIL:])
    head_f = F_TOT - tail_f

    SQ = mybir.ActivationFunctionType.Square
    IS_GT = mybir.AluOpType.is_gt
    MULT = mybir.AluOpType.mult

    n_mm = F_TOT // MM  # total matmul count (per map pair)

    # f16 gram output: halves the final (critical-path) DMA transfer; cell
    # magnitudes ≤ ~10k keep the rounding error ~5e-5 on the losses
    with TileContext(nc) as tc, \
         nc.allow_low_precision(reason="sums fit f16 against the 2e-2 gate"):
        with tc.tile_pool(name="io", bufs=_BUFS[0]) as io, \
             tc.tile_pool(name="mid", bufs=_BUFS[1]) as mid, \
             tc.tile_pool(name="fix", bufs=1) as fix, \
             tc.tile_pool(name="ps", bufs=1, space="PSUM") as ps:
            g_sb = fix.tile([P, W_OUT], _F16)
            st = fix.tile([1, F_TOT + tail_f], _F32)
            split_col = sum(chunks[: nchunk - 3])
            G = ps.tile([P, 3, MM], _F32)

            mm_idx = 0
            off = 0
            for i, f in enumerate(chunks):
                sl = bass.ds(off, f)
                off += f
                # targets first: the mask path (mx) only needs them
                ta_t = io.tile([P, 2, f], _F16, tag="ta")
                nc.sync.dma_start(out=ta_t[:], in_=ta[:, :, sl])
                pa_t = io.tile([P, 2, f], _F16, tag="pa")
                nc.sync.dma_start(out=pa_t[:], in_=pa[:, :, sl])

                # sub first: it unblocks the ACT square, the longest
                # downstream chain
                d = mid.tile([P, 2, f], _F16, tag="d")
                nc.vector.tensor_sub(d[:], pa_t[:], ta_t[:])

                mx = mid.tile([P, f], _F16, tag="mx")
                nc.vector.tensor_max(mx[:], ta_t[:, 0, :], ta_t[:, 1, :])

                msq = mid.tile([P, 3, f], _F16, tag="msq")
                nc.vector.tensor_scalar(msq[:, 0, :], mx[:], 0.5, None, IS_GT)

                if i < nchunk - _DVE_TAIL:
                    nc.scalar.activation(
                        msq[:, 1:3, :], d[:, :, :], SQ,
                        accum_out=g_sb[:, 3 * MM + i : 3 * MM + i + 1],
                    )
                    nc.gpsimd.reduce_sum(
                        st[:, sl], msq[:, 1, :], axis=mybir.AxisListType.C
                    )
                else:
                    t_off = off - f - head_f
                    nc.vector.tensor_mul(msq[:, 1:3, :], d[:, :, :], d[:, :, :])
                    nc.gpsimd.reduce_sum(
                        st[:, F_TOT + t_off : F_TOT + t_off + f], msq[:, 2, :],
                        axis=mybir.AxisListType.C,
                    )
                    nc.gpsimd.reduce_sum(
                        st[:, sl], msq[:, 1, :], axis=mybir.AxisListType.C
                    )
                for s in range(0, f, MM):
                    nc.tensor.matmul(
                        G[:, :, :],
                        msq[:, 0, s : s + MM],
                        msq[:, :, s : s + MM],
                        start=(mm_idx == 0),
                        stop=(mm_idx == n_mm - 1),
                    )
                    mm_idx += 1

            nc.scalar.dma_start(out=st_out[:, :split_col], in_=st[:, :split_col])
            nc.gpsimd.dma_start(out=st_out[:, split_col:], in_=st[:, split_col:])
            nc.vector.tensor_copy(g_sb[:, : 3 * MM], G[:, :, :])
            nc.sync.dma_start(out=g_out[:], in_=g_sb[:])
    nc.compile()
    return nc


_NC_CACHE: dict = {}


def _get_nc() -> bass.Bass:
    if "nc" not in _NC_CACHE:
        _NC_CACHE["nc"] = build_nc()
    return _NC_CACHE["nc"]


def _shard2(r16: np.ndarray, a16: np.ndarray, c: int) -> np.ndarray:
    """Interleave the region/affinity pair for core c as [P, 2, F_TOT]."""
    per_b = B // N_CORES
    r = r16.reshape(B, H * W)[c * per_b : (c + 1) * per_b].reshape(P, F_TOT)
    a = a16.reshape(B, H * W)[c * per_b : (c + 1) * per_b].reshape(P, F_TOT)
    return np.ascontiguousarray(np.stack([r, a], axis=1))


def _core_inputs(np_inputs: dict, c: int) -> dict:
    """Device input map for core c from full-size fp32 inputs (test harness)."""
    return {
        "preds": _shard2(np_inputs["region_pred"].astype(np.float16),
                         np_inputs["affinity_pred"].astype(np.float16), c),
        "targs": _shard2(np_inputs["region_target"].astype(np.float16),
                         np_inputs["affinity_target"].astype(np.float16), c),
    }


def _host_fallback_topk(region_pred, affinity_pred, region_target, affinity_target,
                        n_pos, n_neg):
    """Exact OHEM (reference semantics) on host — unreachable for uniform data."""
    rlm = (region_pred.astype(np.float64) - region_target.astype(np.float64)) ** 2
    alm = (affinity_pred.astype(np.float64) - affinity_target.astype(np.float64)) ** 2
    pos = (region_target > 0.5) | (affinity_target > 0.5)
    neg = ~pos
    comb = ((rlm + alm) * neg).reshape(-1)
    idx = np.argsort(-comb, kind="stable")[:n_neg]
    neg_r = rlm.reshape(-1)[idx].mean()
    neg_a = alm.reshape(-1)[idx].mean()
    pos_r = (rlm * pos).sum() / n_pos
    pos_a = (alm * pos).sum() / n_pos
    return pos_r + neg_r, pos_a + neg_a


def kernel(region_pred, affinity_pred, region_target, affinity_target):
    region_pred = np.asarray(region_pred, dtype=np.float32)
    affinity_pred = np.asarray(affinity_pred, dtype=np.float32)
    region_target = np.asarray(region_target, dtype=np.float32)
    affinity_target = np.asarray(affinity_target, dtype=np.float32)

    rp16 = region_pred.astype(np.float16)
    ap16 = affinity_pred.astype(np.float16)
    rt16 = region_target.astype(np.float16)
    at16 = affinity_target.astype(np.float16)

    nc = _get_nc()
    in_maps = [
        {
            "preds": _shard2(rp16, ap16, c),
            "targs": _shard2(rt16, at16, c),
        }
        for c in range(N_CORES)
    ]
    res = run_bass_kernel_spmd(nc, in_maps, list(range(N_CORES))).results

    chunks = _chunk_list()
    nchunk = len(chunks)
    last_f = sum(chunks[-_DVE_TAIL:])
    S_pos_r = S_pos_a = S_tot_r = S_tot_a = n_pos_f = 0.0
    for c in range(N_CORES):
        out = res[c]["gram"].astype(np.float64)
        g = out[:, : 3 * MM].reshape(P, 3, MM)
        comb = out[:, 3 * MM : 3 * MM + nchunk - _DVE_TAIL].sum()
        stot = res[c]["stot"].astype(np.float64).reshape(-1)
        # stot[:F_TOT] = plane-r colsums (all chunks); stot[F_TOT:] = the
        # final chunk's plane-a colsums; C covers chunks 0..n-2, both planes
        tot_r = stot[:F_TOT].sum()
        tot_a = (comb - stot[: F_TOT - last_f].sum()) + stot[F_TOT:].sum()
        S_tot_r += tot_r
        S_tot_a += tot_a
        n_pos_f += np.trace(g[:, 0, :])
        S_pos_r += np.trace(g[:, 1, :])
        S_pos_a += np.trace(g[:, 2, :])

    n_pos = int(round(n_pos_f))
    n_neg_tot = N_TOTAL - n_pos

    if n_pos == 0:
        region_loss = S_tot_r / N_TOTAL
        affinity_loss = S_tot_a / N_TOTAL
    else:
        pos_r = S_pos_r / n_pos
        pos_a = S_pos_a / n_pos
        n_neg = min(n_neg_tot, int(n_pos * NEG_RATIO))
        if n_neg == 0:
            region_loss, affinity_loss = pos_r, pos_a
        elif n_neg == n_neg_tot:
            region_loss = pos_r + (S_tot_r - S_pos_r) / n_neg
            affinity_loss = pos_a + (S_tot_a - S_pos_a) / n_neg
        else:
            region_loss, affinity_loss = _host_fallback_topk(
                region_pred, affinity_pred, region_target, affinity_target,
                n_pos, n_neg,
            )

    total = np.float32(region_loss + affinity_loss)
    return (total, np.float32(region_loss), np.float32(affinity_loss))


# revision 47
# speedup vs baseline: 1.0145x; 1.0022x over previous
"""CRAFT OHEM loss on 8 trn2 NeuronCores — data-parallel over batch.

Math: with uniform-random inputs, n_neg_total (≈0.25·N) is always far below
NEG_RATIO·n_pos (≈2.25·N), so the reference's OHEM top-k selects *all*
negatives and every branch of the loss reduces to masked global sums:

    pos  = (region_target > .5) | (affinity_target > .5)   [= max(rt,at) > .5]
    S_pos_r = Σ pos·(rp-rt)²    S_tot_r = Σ (rp-rt)²       (same for affinity)
    n_pos   = Σ pos             n_neg_tot = N - n_pos

    region_loss   = S_pos_r/n_pos + (S_tot_r - S_pos_r)/n_neg
    affinity_loss = S_pos_a/n_pos + (S_tot_a - S_pos_a)/n_neg

The kernel streams fp16 inputs (host-side dtype cast; quantization error on
the final losses is ~1e-4, far inside the 2e-2 gate) — halving the HBM
traffic that bounds this kernel.  Preds and targets are host-interleaved as
[P, 2, F] pairs so one DMA / one DVE sub covers both maps.  Per chunk the
engines split the work so every engine hides under the DMA stream
(DMA ~100%, ACT ~70%, DVE ~62%, Pool ~45%, PE ~38%):

    DVE : d = pa − ta  [2x mode], mx = max(ta_r, ta_a) [2x],
          m = (mx > 0.5)  [tensor_scalar, 4x mode]
    ACT : sq = d²  [one instr; accum → per-chunk C = Σ(sq_r+sq_a)]
    Pool: stot[cols] = Σ_p sq_r  [GPSIMD cross-partition reduce]
    PE  : G += mᵀ · [m | sq_r | sq_a]  (PSUM-accumulated Gram; its three
          128×128 diagonal blocks are n_pos / S_pos_r / S_pos_a)

The final (tiny) chunk's squares run on DVE instead of ACT, with extra Pool
reduces covering its sums, so ACT drops out of the end-of-stream drain
chain one chunk early.

Host: S_tot_r = Σstot, S_tot_a = ΣC − S_tot_r, diagonals from G, all in
float64; falls back to an exact numpy OHEM on the original fp32 inputs in
the (unreachable for this input distribution) case
n_neg_tot > NEG_RATIO·n_pos.
"""

import numpy as np

import concourse.bass as bass
import concourse.bacc as bacc
import concourse.mybir as mybir
from concourse.tile import TileContext
from concourse.bass_utils import run_bass_kernel_spmd

N_CORES = 8
B, H, W = 32, 640, 640
N_TOTAL = B * H * W                  # 13_107_200
PER_CORE = N_TOTAL // N_CORES        # 1_638_400
P = 128
F_TOT = PER_CORE // P                # 12_800
NEG_RATIO = 3.0
MM = 128                             # matmul stationary width

_F32 = mybir.dt.float32
_F16 = mybir.dt.float16


_CHUNKS = [512] * 24 + [384, 128]
_BUFS = (6, 4)
_DVE_TAIL = 1          # how many final chunks bypass ACT (DVE squares them)


def _chunk_list(f_tot: int = F_TOT) -> list:
    """Chunks must be multiples of MM=128 (matmul subchunking).  The taper at
    the end shortens the serial drain chain (DVE→ACT→PE) after the last
    input DMA lands."""
    return list(_CHUNKS)


def build_nc() -> bass.Bass:
    chunks = _chunk_list()
    nchunk = len(chunks)
    assert sum(chunks) == F_TOT

    nc = bacc.Bacc(None)
    # preds / targets host-interleaved as [P, 2, F_TOT] (plane 0 = region,
    # plane 1 = affinity): one DMA + one DVE sub covers both maps.
    pa = nc.dram_tensor("preds", [P, 2, F_TOT], _F16, kind="ExternalInput")
    ta = nc.dram_tensor("targs", [P, 2, F_TOT], _F16, kind="ExternalInput")
    # gram: [p, h, j] = Σ_k m[k,p]·[m|sq_r|sq_a][k,h,j] over 128-col
    #       subchunks; the three 128×128 diagonals are n_pos / S_pos_r /
    #       S_pos_a.
    # gram out:
    #   [:, 0:384]          = G (masked Gram, diagonals = n_pos/S_pos_r/a)
    #   [:, 384:384+nchunk] = per-chunk ACT accum C_i = Σ(sq_r+sq_a)
    # stot out: per-column Σ_p sq_r (GPSIMD reduce) — total is S_tot_r;
    #   S_tot_a = ΣC − S_tot_r on the host.
    W_OUT = 3 * MM + nchunk
    g_out = nc.dram_tensor("gram", [P, W_OUT], _F16, kind="ExternalOutput")
    st_out = nc.dram_tensor("stot", [1, F_TOT + sum(chunks[-_DVE_TAIL:])], _F32, kind="ExternalOutput")
    # final _DVE_TAIL chunks skip ACT (DVE squares them); their Σsq_a comes
    # from extra Pool reduces so ACT exits the drain chain early
    tail_f = sum(chunks[-_DVE_TAIL:])
    head_f = F_TOT - tail_f

    SQ = mybir.ActivationFunctionType.Square
    IS_GT = mybir.AluOpType.is_gt
    MULT = mybir.AluOpType.mult

    n_mm = F_TOT // MM  # total matmul count (per map pair)

    # f16 gram output: halves the final (critical-path) DMA transfer; cell
    # magnitudes ≤ ~10k keep the rounding error ~5e-5 on the losses
    with TileContext(nc) as tc, \
         nc.allow_low_precision(reason="sums fit f16 against the 2e-2 gate"):
        with tc.tile_pool(name="io", bufs=_BUFS[0]) as io, \
             tc.tile_pool(name="mid", bufs=_BUFS[1]) as mid, \
             tc.tile_pool(name="fix", bufs=1) as fix, \
             tc.tile_pool(name="ps", bufs=1, space="PSUM") as ps:
            g_sb = fix.tile([P, W_OUT], _F16)
            st = fix.tile([1, F_TOT + tail_f], _F32)
            split_col = sum(chunks[: nchunk - 3])
            G = ps.tile([P, 3, MM], _F32)

            mm_idx = 0
            off = 0
            for i, f in enumerate(chunks):
                sl = bass.ds(off, f)
                off += f
                # targets first: the mask path (mx) only needs them
                ta_t = io.tile([P, 2, f], _F16, tag="ta")
                nc.sync.dma_start(out=ta_t[:], in_=ta[:, :, sl])
                pa_t = io.tile([P, 2, f], _F16, tag="pa")
                nc.sync.dma_start(out=pa_t[:], in_=pa[:, :, sl])

                # sub first: it unblocks the ACT square, the longest
                # downstream chain
                d = mid.tile([P, 2, f], _F16, tag="d")
                nc.vector.tensor_sub(d[:], pa_t[:], ta_t[:])

                mx = mid.tile([P, f], _F16, tag="mx")
                nc.vector.tensor_max(mx[:], ta_t[:, 0, :], ta_t[:, 1, :])

                msq = mid.tile([P, 3, f], _F16, tag="msq")
                nc.vector.tensor_scalar(msq[:, 0, :], mx[:], 0.5, None, IS_GT)

                if i < nchunk - _DVE_TAIL:
                    nc.scalar.activation(
                        msq[:, 1:3, :], d[:, :, :], SQ,
                        accum_out=g_sb[:, 3 * MM + i : 3 * MM + i + 1],
                    )
                    nc.gpsimd.reduce_sum(
                        st[:, sl], msq[:, 1, :], axis=mybir.AxisListType.C
                    )
                else:
                    t_off = off - f - head_f
                    nc.vector.tensor_mul(msq[:, 1:3, :], d[:, :, :], d[:, :, :])
                    nc.gpsimd.reduce_sum(
                        st[:, F_TOT + t_off : F_TOT + t_off + f], msq[:, 2, :],
                        axis=mybir.AxisListType.C,
                    )
                    nc.gpsimd.reduce_sum(
                        st[:, sl], msq[:, 1, :], axis=mybir.AxisListType.C
                    )
                if i == nchunk - 2:
                    deferred = (msq, f)
                    mm_idx += f // MM
                    continue
                for s in range(0, f, MM):
                    nc.tensor.matmul(
                        G[:, :, :],
                        msq[:, 0, s : s + MM],
                        msq[:, :, s : s + MM],
                        start=(mm_idx == 0),
                        stop=False,
                    )
                    mm_idx += 1
            dm, df = deferred
            for s in range(0, df, MM):
                nc.tensor.matmul(
                    G[:, :, :],
                    dm[:, 0, s : s + MM],
                    dm[:, :, s : s + MM],
                    start=False,
                    stop=(s + MM >= df),
                )

            nc.scalar.dma_start(out=st_out[:, :split_col], in_=st[:, :split_col])
            nc.gpsimd.dma_start(out=st_out[:, split_col:], in_=st[:, split_col:])
            nc.vector.tensor_copy(g_sb[:, : 3 * MM], G[:, :, :])
            nc.sync.dma_start(out=g_out[:], in_=g_sb[:])
    nc.compile()
    return nc


_NC_CACHE: dict = {}


def _get_nc() -> bass.Bass:
    if "nc" not in _NC_CACHE:
        _NC_CACHE["nc"] = build_nc()
    return _NC_CACHE["nc"]


def _shard2(r16: np.ndarray, a16: np.ndarray, c: int) -> np.ndarray:
    """Interleave the region/affinity pair for core c as [P, 2, F_TOT]."""
    per_b = B // N_CORES
    r = r16.reshape(B, H * W)[c * per_b : (c + 1) * per_b].reshape(P, F_TOT)
    a = a16.reshape(B, H * W)[c * per_b : (c + 1) * per_b].reshape(P, F_TOT)
    return np.ascontiguousarray(np.stack([r, a], axis=1))


def _core_inputs(np_inputs: dict, c: int) -> dict:
    """Device input map for core c from full-size fp32 inputs (test harness)."""
    return {
        "preds": _shard2(np_inputs["region_pred"].astype(np.float16),
                         np_inputs["affinity_pred"].astype(np.float16), c),
        "targs": _shard2(np_inputs["region_target"].astype(np.float16),
                         np_inputs["affinity_target"].astype(np.float16), c),
    }


def _host_fallback_topk(region_pred, affinity_pred, region_target, affinity_target,
                        n_pos, n_neg):
    """Exact OHEM (reference semantics) on host — unreachable for uniform data."""
    rlm = (region_pred.astype(np.float64) - region_target.astype(np.float64)) ** 2
    alm = (affinity_pred.astype(np.float64) - affinity_target.astype(np.float64)) ** 2
    pos = (region_target > 0.5) | (affinity_target > 0.5)
    neg = ~pos
    comb = ((rlm + alm) * neg).reshape(-1)
    idx = np.argsort(-comb, kind="stable")[:n_neg]
    neg_r = rlm.reshape(-1)[idx].mean()
    neg_a = alm.reshape(-1)[idx].mean()
    pos_r = (rlm * pos).sum() / n_pos
    pos_a = (alm * pos).sum() / n_pos
    return pos_r + neg_r, pos_a + neg_a


def kernel(region_pred, affinity_pred, region_target, affinity_target):
    region_pred = np.asarray(region_pred, dtype=np.float32)
    affinity_pred = np.asarray(affinity_pred, dtype=np.float32)
    region_target = np.asarray(region_target, dtype=np.float32)
    affinity_target = np.asarray(affinity_target, dtype=np.float32)

    rp16 = region_pred.astype(np.float16)
    ap16 = affinity_pred.astype(np.float16)
    rt16 = region_target.astype(np.float16)
    at16 = affinity_target.astype(np.float16)

    nc = _get_nc()
    in_maps = [
        {
            "preds": _shard2(rp16, ap16, c),
            "targs": _shard2(rt16, at16, c),
        }
        for c in range(N_CORES)
    ]
    res = run_bass_kernel_spmd(nc, in_maps, list(range(N_CORES))).results

    chunks = _chunk_list()
    nchunk = len(chunks)
    last_f = sum(chunks[-_DVE_TAIL:])
    S_pos_r = S_pos_a = S_tot_r = S_tot_a = n_pos_f = 0.0
    for c in range(N_CORES):
        out = res[c]["gram"].astype(np.float64)
        g = out[:, : 3 * MM].reshape(P, 3, MM)
        comb = out[:, 3 * MM : 3 * MM + nchunk - _DVE_TAIL].sum()
        stot = res[c]["stot"].astype(np.float64).reshape(-1)
        # stot[:F_TOT] = plane-r colsums (all chunks); stot[F_TOT:] = the
        # final chunk's plane-a colsums; C covers chunks 0..n-2, both planes
        tot_r = stot[:F_TOT].sum()
        tot_a = (comb - stot[: F_TOT - last_f].sum()) + stot[F_TOT:].sum()
        S_tot_r += tot_r
        S_tot_a += tot_a
        n_pos_f += np.trace(g[:, 0, :])
        S_pos_r += np.trace(g[:, 1, :])
        S_pos_a += np.trace(g[:, 2, :])

    n_pos = int(round(n_pos_f))
    n_neg_tot = N_TOTAL - n_pos

    if n_pos == 0:
        region_loss = S_tot_r / N_TOTAL
        affinity_loss = S_tot_a / N_TOTAL
    else:
        pos_r = S_pos_r / n_pos
        pos_a = S_pos_a / n_pos
        n_neg = min(n_neg_tot, int(n_pos * NEG_RATIO))
        if n_neg == 0:
            region_loss, affinity_loss = pos_r, pos_a
        elif n_neg == n_neg_tot:
            region_loss = pos_r + (S_tot_r - S_pos_r) / n_neg
            affinity_loss = pos_a + (S_tot_a - S_pos_a) / n_neg
        else:
            region_loss, affinity_loss = _host_fallback_topk(
                region_pred, affinity_pred, region_target, affinity_target,
                n_pos, n_neg,
            )

    total = np.float32(region_loss + affinity_loss)
    return (total, np.float32(region_loss), np.float32(affinity_loss))


# revision 48
# speedup vs baseline: 1.0169x; 1.0023x over previous
"""CRAFT OHEM loss on 8 trn2 NeuronCores — data-parallel over batch.

Math: with uniform-random inputs, n_neg_total (≈0.25·N) is always far below
NEG_RATIO·n_pos (≈2.25·N), so the reference's OHEM top-k selects *all*
negatives and every branch of the loss reduces to masked global sums:

    pos  = (region_target > .5) | (affinity_target > .5)   [= max(rt,at) > .5]
    S_pos_r = Σ pos·(rp-rt)²    S_tot_r = Σ (rp-rt)²       (same for affinity)
    n_pos   = Σ pos             n_neg_tot = N - n_pos

    region_loss   = S_pos_r/n_pos + (S_tot_r - S_pos_r)/n_neg
    affinity_loss = S_pos_a/n_pos + (S_tot_a - S_pos_a)/n_neg

The kernel streams fp16 inputs (host-side dtype cast; quantization error on
the final losses is ~1e-4, far inside the 2e-2 gate) — halving the HBM
traffic that bounds this kernel.  Preds and targets are host-interleaved as
[P, 2, F] pairs so one DMA / one DVE sub covers both maps.  Per chunk the
engines split the work so every engine hides under the DMA stream
(DMA ~100%, ACT ~70%, DVE ~62%, Pool ~45%, PE ~38%):

    DVE : d = pa − ta  [2x mode], mx = max(ta_r, ta_a) [2x],
          m = (mx > 0.5)  [tensor_scalar, 4x mode]
    ACT : sq = d²  [one instr; accum → per-chunk C = Σ(sq_r+sq_a)]
    Pool: stot[cols] = Σ_p sq_r  [GPSIMD cross-partition reduce]
    PE  : G += mᵀ · [m | sq_r | sq_a]  (PSUM-accumulated Gram; its three
          128×128 diagonal blocks are n_pos / S_pos_r / S_pos_a)

The final (tiny) chunk's squares run on DVE instead of ACT, with extra Pool
reduces covering its sums, so ACT drops out of the end-of-stream drain
chain one chunk early.

Host: S_tot_r = Σstot, S_tot_a = ΣC − S_tot_r, diagonals from G, all in
float64; falls back to an exact numpy OHEM on the original fp32 inputs in
the (unreachable for this input distribution) case
n_neg_tot > NEG_RATIO·n_pos.
"""

import numpy as np

import concourse.bass as bass
import concourse.bacc as bacc
import concourse.mybir as mybir
from concourse.tile import TileContext
from concourse.bass_utils import run_bass_kernel_spmd

N_CORES = 8
B, H, W = 32, 640, 640
N_TOTAL = B * H * W                  # 13_107_200
PER_CORE = N_TOTAL // N_CORES        # 1_638_400
P = 128
F_TOT = PER_CORE // P                # 12_800
NEG_RATIO = 3.0
MM = 128                             # matmul stationary width

_F32 = mybir.dt.float32
_F16 = mybir.dt.float16


_CHUNKS = [512] * 24 + [384, 128]
_BUFS = (6, 4)
_DVE_TAIL = 1          # how many final chunks bypass ACT (DVE squares them)


def _chunk_list(f_tot: int = F_TOT) -> list:
    """Chunks must be multiples of MM=128 (matmul subchunking).  The taper at
    the end shortens the serial drain chain (DVE→ACT→PE) after the last
    input DMA lands."""
    return list(_CHUNKS)


def build_nc() -> bass.Bass:
    chunks = _chunk_list()
    nchunk = len(chunks)
    assert sum(chunks) == F_TOT

    nc = bacc.Bacc(None)
    # preds / targets host-interleaved as [P, 2, F_TOT] (plane 0 = region,
    # plane 1 = affinity): one DMA + one DVE sub covers both maps.
    pa = nc.dram_tensor("preds", [P, 2, F_TOT], _F16, kind="ExternalInput")
    ta = nc.dram_tensor("targs", [P, 2, F_TOT], _F16, kind="ExternalInput")
    # gram: [p, h, j] = Σ_k m[k,p]·[m|sq_r|sq_a][k,h,j] over 128-col
    #       subchunks; the three 128×128 diagonals are n_pos / S_pos_r /
    #       S_pos_a.
    # gram out:
    #   [:, 0:384]          = G (masked Gram, diagonals = n_pos/S_pos_r/a)
    #   [:, 384:384+nchunk] = per-chunk ACT accum C_i = Σ(sq_r+sq_a)
    # stot out: per-column Σ_p sq_r (GPSIMD reduce) — total is S_tot_r;
    #   S_tot_a = ΣC − S_tot_r on the host.
    W_OUT = 3 * MM + nchunk
    g_out = nc.dram_tensor("gram", [P, W_OUT], _F16, kind="ExternalOutput")
    st_out = nc.dram_tensor("stot", [1, F_TOT + sum(chunks[-_DVE_TAIL:])], _F32, kind="ExternalOutput")
    # final _DVE_TAIL chunks skip ACT (DVE squares them); their Σsq_a comes
    # from extra Pool reduces so ACT exits the drain chain early
    tail_f = sum(chunks[-_DVE_TAIL:])
    head_f = F_TOT - tail_f

    SQ = mybir.ActivationFunctionType.Square
    IS_GT = mybir.AluOpType.is_gt
    MULT = mybir.AluOpType.mult

    n_mm = F_TOT // MM  # total matmul count (per map pair)

    # f16 gram output: halves the final (critical-path) DMA transfer; cell
    # magnitudes ≤ ~10k keep the rounding error ~5e-5 on the losses
    with TileContext(nc) as tc, \
         nc.allow_low_precision(reason="sums fit f16 against the 2e-2 gate"):
        with tc.tile_pool(name="io", bufs=_BUFS[0]) as io, \
             tc.tile_pool(name="mid", bufs=_BUFS[1]) as mid, \
             tc.tile_pool(name="fix", bufs=1) as fix, \
             tc.tile_pool(name="ps", bufs=1, space="PSUM") as ps:
            g_sb = fix.tile([P, W_OUT], _F16)
            st = fix.tile([1, F_TOT + tail_f], _F32)
            split_col = sum(chunks[: nchunk - 3])
            G = ps.tile([P, 3, MM], _F32)

            mm_idx = 0
            off = 0
            for i, f in enumerate(chunks):
                sl = bass.ds(off, f)
                off += f
                # targets first: the mask path (mx) only needs them
                ta_t = io.tile([P, 2, f], _F16, tag="ta")
                nc.sync.dma_start(out=ta_t[:], in_=ta[:, :, sl])
                pa_t = io.tile([P, 2, f], _F16, tag="pa")
                nc.sync.dma_start(out=pa_t[:], in_=pa[:, :, sl])

                # sub first: it unblocks the ACT square, the longest
                # downstream chain
                d = mid.tile([P, 2, f], _F16, tag="d")
                nc.vector.tensor_sub(d[:], pa_t[:], ta_t[:])

                mx = mid.tile([P, f], _F16, tag="mx")
                nc.vector.tensor_max(mx[:], ta_t[:, 0, :], ta_t[:, 1, :])

                msq = mid.tile([P, 3, f], _F16, tag="msq")
                nc.vector.tensor_scalar(msq[:, 0, :], mx[:], 0.5, None, IS_GT)

                if i < nchunk - _DVE_TAIL:
                    nc.scalar.activation(
                        msq[:, 1:3, :], d[:, :, :], SQ,
                        accum_out=g_sb[:, 3 * MM + i : 3 * MM + i + 1],
                    )
                    nc.gpsimd.reduce_sum(
                        st[:, sl], msq[:, 1, :], axis=mybir.AxisListType.C
                    )
                else:
                    t_off = off - f - head_f
                    nc.vector.tensor_mul(msq[:, 1:3, :], d[:, :, :], d[:, :, :])
                    nc.gpsimd.reduce_sum(
                        st[:, F_TOT + t_off : F_TOT + t_off + f], msq[:, 2, :],
                        axis=mybir.AxisListType.C,
                    )
                    nc.gpsimd.reduce_sum(
                        st[:, sl], msq[:, 1, :], axis=mybir.AxisListType.C
                    )
                if i == nchunk - 2:
                    deferred = (msq, f)
                    mm_idx += f // MM
                    continue
                for s in range(0, f, MM):
                    nc.tensor.matmul(
                        G[:, :, :],
                        msq[:, 0, s : s + MM],
                        msq[:, :, s : s + MM],
                        start=(mm_idx == 0),
                        stop=False,
                    )
                    mm_idx += 1
            dm, df = deferred
            for s in range(0, df, MM):
                nc.tensor.matmul(
                    G[:, 0:1, :],
                    dm[:, 0, s : s + MM],
                    dm[:, 0:1, s : s + MM],
                    start=False,
                    stop=(s + MM >= df),
                    skip_group_check=True,
                )
            for s in range(0, df, MM):
                nc.tensor.matmul(
                    G[:, 1:3, :],
                    dm[:, 0, s : s + MM],
                    dm[:, 1:3, s : s + MM],
                    start=False,
                    stop=(s + MM >= df),
                    skip_group_check=True,
                )

            nc.scalar.dma_start(out=st_out[:, :split_col], in_=st[:, :split_col])
            nc.gpsimd.dma_start(out=st_out[:, split_col:], in_=st[:, split_col:])
            nc.vector.tensor_copy(g_sb[:, : 3 * MM], G[:, :, :])
            nc.sync.dma_start(out=g_out[:], in_=g_sb[:])
    nc.compile()
    return nc


_NC_CACHE: dict = {}


def _get_nc() -> bass.Bass:
    if "nc" not in _NC_CACHE:
        _NC_CACHE["nc"] = build_nc()
    return _NC_CACHE["nc"]


def _shard2(r16: np.ndarray, a16: np.ndarray, c: int) -> np.ndarray:
    """Interleave the region/affinity pair for core c as [P, 2, F_TOT]."""
    per_b = B // N_CORES
    r = r16.reshape(B, H * W)[c * per_b : (c + 1) * per_b].reshape(P, F_TOT)
    a = a16.reshape(B, H * W)[c * per_b : (c + 1) * per_b].reshape(P, F_TOT)
    return np.ascontiguousarray(np.stack([r, a], axis=1))


def _core_inputs(np_inputs: dict, c: int) -> dict:
    """Device input map for core c from full-size fp32 inputs (test harness)."""
    return {
        "preds": _shard2(np_inputs["region_pred"].astype(np.float16),
                         np_inputs["affinity_pred"].astype(np.float16), c),
        "targs": _shard2(np_inputs["region_target"].astype(np.float16),
                         np_inputs["affinity_target"].astype(np.float16), c),
    }


def _host_fallback_topk(region_pred, affinity_pred, region_target, affinity_target,
                        n_pos, n_neg):
    """Exact OHEM (reference semantics) on host — unreachable for uniform data."""
    rlm = (region_pred.astype(np.float64) - region_target.astype(np.float64)) ** 2
    alm = (affinity_pred.astype(np.float64) - affinity_target.astype(np.float64)) ** 2
    pos = (region_target > 0.5) | (affinity_target > 0.5)
    neg = ~pos
    comb = ((rlm + alm) * neg).reshape(-1)
    idx = np.argsort(-comb, kind="stable")[:n_neg]
    neg_r = rlm.reshape(-1)[idx].mean()
    neg_a = alm.reshape(-1)[idx].mean()
    pos_r = (rlm * pos).sum() / n_pos
    pos_a = (alm * pos).sum() / n_pos
    return pos_r + neg_r, pos_a + neg_a


def kernel(region_pred, affinity_pred, region_target, affinity_target):
    region_pred = np.asarray(region_pred, dtype=np.float32)
    affinity_pred = np.asarray(affinity_pred, dtype=np.float32)
    region_target = np.asarray(region_target, dtype=np.float32)
    affinity_target = np.asarray(affinity_target, dtype=np.float32)

    rp16 = region_pred.astype(np.float16)
    ap16 = affinity_pred.astype(np.float16)
    rt16 = region_target.astype(np.float16)
    at16 = affinity_target.astype(np.float16)

    nc = _get_nc()
    in_maps = [
        {
            "preds": _shard2(rp16, ap16, c),
            "targs": _shard2(rt16, at16, c),
        }
        for c in range(N_CORES)
    ]
    res = run_bass_kernel_spmd(nc, in_maps, list(range(N_CORES))).results

    chunks = _chunk_list()
    nchunk = len(chunks)
    last_f = sum(chunks[-_DVE_TAIL:])
    S_pos_r = S_pos_a = S_tot_r = S_tot_a = n_pos_f = 0.0
    for c in range(N_CORES):
        out = res[c]["gram"].astype(np.float64)
        g = out[:, : 3 * MM].reshape(P, 3, MM)
        comb = out[:, 3 * MM : 3 * MM + nchunk - _DVE_TAIL].sum()
        stot = res[c]["stot"].astype(np.float64).reshape(-1)
        # stot[:F_TOT] = plane-r colsums (all chunks); stot[F_TOT:] = the
        # final chunk's plane-a colsums; C covers chunks 0..n-2, both planes
        tot_r = stot[:F_TOT].sum()
        tot_a = (comb - stot[: F_TOT - last_f].sum()) + stot[F_TOT:].sum()
        S_tot_r += tot_r
        S_tot_a += tot_a
        n_pos_f += np.trace(g[:, 0, :])
        S_pos_r += np.trace(g[:, 1, :])
        S_pos_a += np.trace(g[:, 2, :])

    n_pos = int(round(n_pos_f))
    n_neg_tot = N_TOTAL - n_pos

    if n_pos == 0:
        region_loss = S_tot_r / N_TOTAL
        affinity_loss = S_tot_a / N_TOTAL
    else:
        pos_r = S_pos_r / n_pos
        pos_a = S_pos_a / n_pos
        n_neg = min(n_neg_tot, int(n_pos * NEG_RATIO))
        if n_neg == 0:
            region_loss, affinity_loss = pos_r, pos_a
        elif n_neg == n_neg_tot:
            region_loss = pos_r + (S_tot_r - S_pos_r) / n_neg
            affinity_loss = pos_a + (S_tot_a - S_pos_a) / n_neg
        else:
            region_loss, affinity_loss = _host_fallback_topk(
                region_pred, affinity_pred, region_target, affinity_target,
                n_pos, n_neg,
            )

    total = np.float32(region_loss + affinity_loss)
    return (total, np.float32(region_loss), np.float32(affinity_loss))
